# revision 1
# baseline (speedup 1.0000x reference)
"""Trainium2 Bass kernel for nn_CustomLoss_188978561648.

loss = -(1/K) * sum_{k,i} num[k,i] / (var + rs[k,i] - num[k,i])
  rs  = zs @ X.T          [K, N]   (the dominant GEMM)
  num = zs * diag(X)      [K, N]

Sharding: tensor-parallel over the output columns i (rows of X).
Core c owns i in [c*512, (c+1)*512): it loads X.T[:, shard] (8 MB),
the full zs (transposed, 1 MB), computes rs[:, shard] with 32
accumulating matmuls (contraction n on the partition axis), runs the
elementwise epilogue + free-axis reduction on DVE, and emits a [K, 1]
partial, already scaled by -1/K. Host unshard = sum of the 8 partials.

X is transposed/packed on the host so every DMA is a contiguous 1 MB
block and the contraction dim lands on SBUF partitions. Matmuls run in
float32r (fp32 data, 1 cycle/row at N=512 vs 4 for plain fp32), which
keeps the kernel on the DMA roofline (~8 MB/core @ ~360 GB/s).
"""

import numpy as np

K = 64          # schedules (zs rows)
N = 4096        # channel dim
NCORES = 8
SHARD = N // NCORES            # 512 output columns per core
NCHUNKS = N // 128             # 32 contraction chunks of 128
GROUPS = 8                     # xt DMA groups per core
CPG = NCHUNKS // GROUPS        # 4 chunks (matmuls) per DMA group

_CACHE = {}


def _build(mm_dtype_name="float32r"):
    import concourse.bacc as bacc
    import concourse.tile as tile
    import concourse.mybir as mybir

    f32 = mybir.dt.float32
    fmm = getattr(mybir.dt, mm_dtype_name)

    nc = bacc.Bacc(
        "TRN2", target_bir_lowering=False, debug=False, num_devices=NCORES
    )

    xt_d = nc.dram_tensor("xt", [GROUPS, 128, CPG * SHARD], fmm, kind="ExternalInput")
    zst_d = nc.dram_tensor("zst", [128, NCHUNKS * K], fmm, kind="ExternalInput")
    zs_d = nc.dram_tensor("zs_sh", [K, SHARD], f32, kind="ExternalInput")
    diag_d = nc.dram_tensor("diag", [K, SHARD], f32, kind="ExternalInput")
    var_d = nc.dram_tensor("var", [K, 1], f32, kind="ExternalInput")
    out_d = nc.dram_tensor("out", [K, 1], f32, kind="ExternalOutput")

    with tile.TileContext(nc) as tc:
        with (
            tc.tile_pool(name="consts", bufs=1) as cpool,
            tc.tile_pool(name="xt", bufs=4) as xpool,
            tc.tile_pool(name="ep", bufs=1) as epool,
            tc.tile_pool(name="ps", bufs=1, space="PSUM") as pspool,
        ):
            zst_t = cpool.tile([128, NCHUNKS * K], fmm)
            nc.sync.dma_start(zst_t[:], zst_d[:])
            zs_t = cpool.tile([K, SHARD], f32)
            nc.sync.dma_start(zs_t[:], zs_d[:])
            diag_t = cpool.tile([K, SHARD], f32)
            nc.sync.dma_start(diag_t[:], diag_d[:])
            var_t = cpool.tile([K, 1], f32)
            nc.sync.dma_start(var_t[:], var_d[:])

            ps = pspool.tile([K, SHARD], f32)
            for g in range(GROUPS):
                xt_t = xpool.tile([128, CPG * SHARD], fmm, name=f"xt_{g}", tag="xt")
                nc.sync.dma_start(xt_t[:], xt_d[g, :, :])
                for j in range(CPG):
                    m = g * CPG + j
                    nc.tensor.matmul(
                        ps[:],
                        zst_t[:, m * K : (m + 1) * K],
                        xt_t[:, j * SHARD : (j + 1) * SHARD],
                        start=(m == 0),
                        stop=(m == NCHUNKS - 1),
                    )

            num_t = epool.tile([K, SHARD], f32)
            nc.vector.tensor_tensor(
                num_t[:], zs_t[:], diag_t[:], op=mybir.AluOpType.mult
            )
            den_t = epool.tile([K, SHARD], f32)
            nc.vector.tensor_scalar_add(den_t[:], ps[:], var_t[:])
            nc.vector.tensor_tensor(
                den_t[:], den_t[:], num_t[:], op=mybir.AluOpType.subtract
            )
            rcp_t = epool.tile([K, SHARD], f32)
            nc.vector.reciprocal(rcp_t[:], den_t[:])
            q_t = epool.tile([K, SHARD], f32)
            nc.vector.tensor_tensor(
                q_t[:], num_t[:], rcp_t[:], op=mybir.AluOpType.mult
            )
            red_t = epool.tile([K, 1], f32)
            nc.vector.tensor_reduce(
                red_t[:], q_t[:], axis=mybir.AxisListType.X, op=mybir.AluOpType.add
            )
            outs_t = epool.tile([K, 1], f32)
            nc.scalar.mul(outs_t[:], red_t[:], -1.0 / K)
            nc.sync.dma_start(out_d[:], outs_t[:])

    nc.compile()
    return nc


def _prep_inputs(zs, X, var_noise):
    """Host-side shard + layout packing (pure layout, no math beyond
    extracting diag(X))."""
    zs = np.ascontiguousarray(np.asarray(zs, dtype=np.float32))
    X = np.ascontiguousarray(np.asarray(X, dtype=np.float32))
    var = np.float32(np.asarray(var_noise).reshape(()))

    # xt_packed[c, g, p, j*SHARD + il] = X[c*SHARD + il, (g*CPG + j)*128 + p]
    xt_packed = np.ascontiguousarray(
        X.reshape(NCORES, SHARD, GROUPS, CPG, 128).transpose(0, 2, 4, 3, 1)
    ).reshape(NCORES, GROUPS, 128, CPG * SHARD)

    # zst_packed[p, m*K + k] = zs[k, m*128 + p]   (replicated to all cores)
    zst_packed = np.ascontiguousarray(
        zs.reshape(K, NCHUNKS, 128).transpose(2, 1, 0)
    ).reshape(128, NCHUNKS * K)

    diag = np.ascontiguousarray(np.diagonal(X))
    var_tile = np.full((K, 1), var, dtype=np.float32)

    in_maps = []
    for c in range(NCORES):
        sl = slice(c * SHARD, (c + 1) * SHARD)
        in_maps.append(
            {
                "xt": xt_packed[c],
                "zst": zst_packed,
                "zs_sh": np.ascontiguousarray(zs[:, sl]),
                "diag": np.ascontiguousarray(
                    np.broadcast_to(diag[sl], (K, SHARD))
                ),
                "var": var_tile,
            }
        )
    return in_maps


def _run(in_maps, mm_dtype_name="float32r", **run_kwargs):
    from concourse.bass_utils import run_bass_kernel_spmd

    key = ("nc", mm_dtype_name)
    if key not in _CACHE:
        _CACHE[key] = _build(mm_dtype_name)
    nc = _CACHE[key]
    return run_bass_kernel_spmd(
        nc, in_maps, core_ids=list(range(NCORES)), **run_kwargs
    )


def kernel(zs, X, var_noise):
    in_maps = _prep_inputs(zs, X, var_noise)
    res = _run(in_maps).results
    total = np.float32(0.0)
    for c in range(NCORES):
        total += res[c]["out"].astype(np.float32).sum(dtype=np.float32)
    return np.float32(total)


# revision 2
# speedup vs baseline: 1.4564x; 1.4564x over previous
"""Trainium2 Bass kernel for nn_CustomLoss_188978561648.

loss = -(1/K) * sum_{k,i} num[k,i] / (var + rs[k,i] - num[k,i])
  rs  = zs @ X.T          [K, N]   (the dominant GEMM)
  num = zs * diag(X)      [K, N]

Sharding: tensor-parallel over the output columns i (rows of X).
Core c owns i in [c*512, (c+1)*512): it loads X.T[:, shard], the full
zs (transposed), computes rs[:, shard] with 32 accumulating matmuls
(contraction n on the partition axis), runs the elementwise epilogue +
free-axis reduction on DVE, and emits a [128, 1] partial, already
scaled by -1/K. Host unshard = sum of the 8 partials.

Perf notes:
- X/zs matmul operands are cast to fp16 on the host: the loss changes
  by ~5e-7 relative (fp32 PSUM accumulation; num/den still use fp32
  data), and DMA bytes halve -> the kernel rides the ~360 GB/s HBM
  roofline at ~4.4 MB/core.
- X is transposed/packed on the host so every DMA is a contiguous
  512 KB block with the contraction dim on SBUF partitions.
- xt stream goes on the Sync HWDGE ring; zst + epilogue tensors ride
  the Scalar HWDGE ring so they don't serialize behind the stream.
- Epilogue reshapes [64, 512] PSUM into [128, 256] tiles (full DVE
  width) and uses the 2-ULP approx reciprocal instead of the exact
  one (3.3 us -> ~0.5 us).
"""

import numpy as np

K = 64          # schedules (zs rows)
N = 4096        # channel dim
NCORES = 8
SHARD = N // NCORES            # 512 output columns per core
NCHUNKS = N // 128             # 32 contraction chunks of 128
GROUPS = 8                     # xt DMA groups per core
CPG = NCHUNKS // GROUPS        # 4 chunks (matmuls) per DMA group
EP = SHARD // 2                # 256: epilogue free size at 128 partitions

_CACHE = {}


def _build(mm_dtype_name="float16"):
    import concourse.bacc as bacc
    import concourse.tile as tile
    import concourse.mybir as mybir

    f32 = mybir.dt.float32
    fmm = getattr(mybir.dt, mm_dtype_name)

    nc = bacc.Bacc(
        "TRN2", target_bir_lowering=False, debug=False, num_devices=NCORES
    )

    xt_d = nc.dram_tensor("xt", [GROUPS, 128, CPG * SHARD], fmm, kind="ExternalInput")
    zst_d = nc.dram_tensor("zst", [128, NCHUNKS * K], fmm, kind="ExternalInput")
    zs_d = nc.dram_tensor("zs_sh", [128, EP], f32, kind="ExternalInput")
    diag_d = nc.dram_tensor("diag", [128, EP], f32, kind="ExternalInput")
    var_d = nc.dram_tensor("var", [128, 1], f32, kind="ExternalInput")
    out_d = nc.dram_tensor("out", [128, 1], f32, kind="ExternalOutput")

    with tile.TileContext(nc) as tc:
        with (
            tc.tile_pool(name="consts", bufs=1) as cpool,
            tc.tile_pool(name="xt", bufs=4) as xpool,
            tc.tile_pool(name="ep", bufs=1) as epool,
            tc.tile_pool(name="ps", bufs=1, space="PSUM") as pspool,
        ):
            # constants ride the Scalar HWDGE ring; the xt stream owns Sync's
            zst_t = cpool.tile([128, NCHUNKS * K], fmm)
            nc.scalar.dma_start(zst_t[:], zst_d[:])
            zs_t = cpool.tile([128, EP], f32)
            nc.scalar.dma_start(zs_t[:], zs_d[:])
            diag_t = cpool.tile([128, EP], f32)
            nc.scalar.dma_start(diag_t[:], diag_d[:])
            var_t = cpool.tile([128, 1], f32)
            nc.scalar.dma_start(var_t[:], var_d[:])

            ps = pspool.tile([K, SHARD], f32)
            for g in range(GROUPS):
                xt_t = xpool.tile([128, CPG * SHARD], fmm, name=f"xt_{g}", tag="xt")
                nc.sync.dma_start(xt_t[:], xt_d[g, :, :])
                for j in range(CPG):
                    m = g * CPG + j
                    nc.tensor.matmul(
                        ps[:],
                        zst_t[:, m * K : (m + 1) * K],
                        xt_t[:, j * SHARD : (j + 1) * SHARD],
                        start=(m == 0),
                        stop=(m == NCHUNKS - 1),
                    )

            # epilogue at [128, EP]: partition p<64 -> (k=p, i in [0,EP)),
            # p>=64 -> (k=p-64, i in [EP, SHARD))
            den_t = epool.tile([128, EP], f32)
            nc.vector.tensor_scalar_add(den_t[:K, :], ps[:, :EP], var_t[:K])
            nc.vector.tensor_scalar_add(den_t[K:, :], ps[:, EP:], var_t[K:])
            num_t = epool.tile([128, EP], f32)
            nc.vector.tensor_tensor(
                num_t[:], zs_t[:], diag_t[:], op=mybir.AluOpType.mult
            )
            nc.vector.tensor_tensor(
                den_t[:], den_t[:], num_t[:], op=mybir.AluOpType.subtract
            )
            rcp_t = epool.tile([128, EP], f32)
            scr_t = epool.tile([128, EP], f32)
            nc.vector.reciprocal_approx_accurate(rcp_t[:], den_t[:], scr_t[:])
            nc.vector.tensor_tensor(
                rcp_t[:], num_t[:], rcp_t[:], op=mybir.AluOpType.mult
            )
            red_t = epool.tile([128, 1], f32)
            nc.vector.tensor_reduce(
                red_t[:], rcp_t[:], axis=mybir.AxisListType.X, op=mybir.AluOpType.add
            )
            outs_t = epool.tile([128, 1], f32)
            nc.vector.tensor_scalar_mul(outs_t[:], red_t[:], -1.0 / K)
            nc.scalar.dma_start(out_d[:], outs_t[:])

    nc.compile()
    return nc


def _prep_inputs(zs, X, var_noise, mm_dtype_name="float16"):
    """Host-side shard + layout packing (layout + dtype cast only; the
    only math is extracting diag(X))."""
    np_mm = {"float16": np.float16, "bfloat16": None, "float32r": np.float32,
             "float32": np.float32}[mm_dtype_name]
    if np_mm is None:
        import ml_dtypes
        np_mm = ml_dtypes.bfloat16
    zs = np.ascontiguousarray(np.asarray(zs, dtype=np.float32))
    X = np.ascontiguousarray(np.asarray(X, dtype=np.float32))
    var = np.float32(np.asarray(var_noise).reshape(()))

    # xt_packed[c, g, p, j*SHARD + il] = X[c*SHARD + il, (g*CPG + j)*128 + p]
    xt_packed = np.ascontiguousarray(
        X.reshape(NCORES, SHARD, GROUPS, CPG, 128)
        .transpose(0, 2, 4, 3, 1)
        .astype(np_mm)
    ).reshape(NCORES, GROUPS, 128, CPG * SHARD)

    # zst_packed[p, m*K + k] = zs[k, m*128 + p]   (replicated to all cores)
    zst_packed = np.ascontiguousarray(
        zs.reshape(K, NCHUNKS, 128).transpose(2, 1, 0).astype(np_mm)
    ).reshape(128, NCHUNKS * K)

    diag = np.ascontiguousarray(np.diagonal(X))
    var_tile = np.full((128, 1), var, dtype=np.float32)

    def fold(a):  # [K, SHARD] -> [128, EP] epilogue layout
        return np.ascontiguousarray(np.concatenate([a[:, :EP], a[:, EP:]], axis=0))

    in_maps = []
    for c in range(NCORES):
        sl = slice(c * SHARD, (c + 1) * SHARD)
        zs_sh = zs[:, sl]
        diag_bc = np.broadcast_to(diag[sl], (K, SHARD))
        in_maps.append(
            {
                "xt": xt_packed[c],
                "zst": zst_packed,
                "zs_sh": fold(zs_sh),
                "diag": fold(diag_bc),
                "var": var_tile,
            }
        )
    return in_maps


def _run(in_maps, mm_dtype_name="float16", **run_kwargs):
    from concourse.bass_utils import run_bass_kernel_spmd

    key = ("nc", mm_dtype_name)
    if key not in _CACHE:
        _CACHE[key] = _build(mm_dtype_name)
    nc = _CACHE[key]
    return run_bass_kernel_spmd(
        nc, in_maps, core_ids=list(range(NCORES)), **run_kwargs
    )


def kernel(zs, X, var_noise):
    in_maps = _prep_inputs(zs, X, var_noise)
    res = _run(in_maps).results
    total = np.float32(0.0)
    for c in range(NCORES):
        total += res[c]["out"].astype(np.float32).sum(dtype=np.float32)
    return np.float32(total)


# revision 10
# speedup vs baseline: 1.6526x; 1.1348x over previous
"""Trainium2 Bass kernel for nn_CustomLoss_188978561648.

loss = -(1/K) * sum_{k,i} num[k,i] / (var + rs[k,i] - num[k,i])
  rs  = zs @ X.T          [K, N]   (the dominant GEMM)
  num = zs * diag(X)      [K, N]

Sharding: tensor-parallel over the output columns i (rows of X).
Core c owns i in [c*512, (c+1)*512): it loads X.T[:, shard], the full
zs (transposed), computes rs[:, shard] with 32 accumulating matmuls
(contraction n on the partition axis), runs the fused elementwise
epilogue + reduction on DVE, cross-partition-reduces on the PE with a
ones vector, and emits one fp32 scalar already scaled by -1/K.
Host unshard = sum of the 8 per-core scalars.

Perf notes (measured on HW):
- X/zs matmul operands are cast to fp16 on the host: the loss changes
  by ~5e-7 relative (fp32 PSUM accumulation; num/den still use fp32
  data), and DMA bytes halve -> the kernel rides the ~360 GB/s HBM
  roofline at ~4.7 MB/core.
- X is transposed/packed on the host so every DMA is a contiguous
  block with the contraction dim on SBUF partitions.
- The xt/zst stream is interleaved across BOTH HWDGE rings (Sync +
  Scalar engines) in FIFO order zst_g before xt_g, so matmul group g
  is gated only by its own bytes; the small epilogue tensors ride
  mid-stream on the Scalar ring.
- Dummy matmuls on memset data warm the PE HAM clock gate during the
  initial DMA fill so real matmuls run at full clock.
- Epilogue at [128, 256] (full DVE width), 2-ULP approx reciprocal,
  scalar_tensor_tensor + tensor_tensor_reduce fusions; the final
  output is a [1, 1] scalar so its DMA uses a single engine (a
  [128, 1] output pays ~16 straggling per-engine sem completions).
"""

import numpy as np

K = 64          # schedules (zs rows)
N = 4096        # channel dim
NCORES = 8
SHARD = N // NCORES            # 512 output columns per core
NCHUNKS = N // 128             # 32 contraction chunks of 128
GROUPS = 8                     # xt DMA groups per core
CPG = NCHUNKS // GROUPS        # 4 chunks (matmuls) per DMA group
EP = SHARD // 2                # 256: epilogue free size at 128 partitions
N_WARM = 12                    # PE warm-up dummy matmuls
WARM_ROWS = 256

_CACHE = {}


def _build(mm_dtype_name="float16", warm=N_WARM, fin="pe", ep="stt"):
    import concourse.bacc as bacc
    import concourse.tile as tile
    import concourse.mybir as mybir

    f32 = mybir.dt.float32
    fmm = getattr(mybir.dt, mm_dtype_name)

    nc = bacc.Bacc(
        "TRN2", target_bir_lowering=False, debug=False, num_devices=NCORES
    )

    xt_d = nc.dram_tensor("xt", [GROUPS, 128, CPG * SHARD], fmm, kind="ExternalInput")
    zst_d = nc.dram_tensor("zst", [GROUPS, 128, CPG * K], fmm, kind="ExternalInput")
    zs_d = nc.dram_tensor("zs_sh", [128, EP], f32, kind="ExternalInput")
    diag_d = nc.dram_tensor("diag", [128, EP], f32, kind="ExternalInput")
    var_d = nc.dram_tensor("var", [128, 1], f32, kind="ExternalInput")
    out_shape = [1, 1] if fin == "pe" else [128, 1]
    out_d = nc.dram_tensor("out", out_shape, f32, kind="ExternalOutput")

    with tile.TileContext(nc) as tc:
        with (
            tc.tile_pool(name="data", bufs=1) as dpool,
            tc.tile_pool(name="ep", bufs=1) as epool,
            tc.tile_pool(name="ps", bufs=1, space="PSUM") as pspool,
        ):
            # -- PE warm-up fodder (no DMA inputs) --
            dw_t = dpool.tile([128, WARM_ROWS], fmm, tag="dw")
            nc.vector.memset(dw_t[:], 0.0)
            ones_t = dpool.tile([128, 1], f32, tag="ones")
            nc.vector.memset(ones_t[:], 1.0)

            # -- stream DMAs: interleave zst_g before xt_g across both
            #    HWDGE rings (Sync: even groups, Scalar: odd groups) --
            zst_t = [
                dpool.tile([128, CPG * K], fmm, name=f"zst{g}", tag=f"zst{g}")
                for g in range(GROUPS)
            ]
            xt_t = [
                dpool.tile([128, CPG * SHARD], fmm, name=f"xt{g}", tag=f"xt{g}")
                for g in range(GROUPS)
            ]
            zs_t = epool.tile([128, EP], f32, tag="zs")
            diag_t = epool.tile([128, EP], f32, tag="diag")
            var_t = epool.tile([128, 1], f32, tag="var")

            ring = {0: nc.sync, 1: nc.scalar}
            for g in range(GROUPS):
                eng = ring[g % 2]
                eng.dma_start(zst_t[g][:], zst_d[g, :, :])
                eng.dma_start(xt_t[g][:], xt_d[g, :, :])
                if g == 5:
                    # epilogue tensors ride mid-stream on the scalar ring
                    nc.scalar.dma_start(zs_t[:], zs_d[:])
                    nc.scalar.dma_start(diag_t[:], diag_d[:])
                    nc.scalar.dma_start(var_t[:], var_d[:])

            # -- PE: warm-up dummies, then the 32-chunk accumulation --
            if warm:
                dummy_ps = pspool.tile([K, WARM_ROWS], f32, tag="dummy_ps")
                for w in range(warm):
                    nc.tensor.matmul(
                        dummy_ps[:], dw_t[:, :K], dw_t[:], start=True, stop=True
                    )

            ps = pspool.tile([K, SHARD], f32, tag="ps")
            for g in range(GROUPS):
                for j in range(CPG):
                    m = g * CPG + j
                    nc.tensor.matmul(
                        ps[:],
                        zst_t[g][:, j * K : (j + 1) * K],
                        xt_t[g][:, j * SHARD : (j + 1) * SHARD],
                        start=(m == 0),
                        stop=(m == NCHUNKS - 1),
                    )

            # -- epilogue at [128, EP]: partition p<64 -> (k=p, i<EP),
            #    p>=64 -> (k=p-64, i>=EP) --
            num_t = epool.tile([128, EP], f32, tag="num")
            nc.vector.tensor_tensor(
                num_t[:], zs_t[:], diag_t[:], op=mybir.AluOpType.mult
            )
            den_t = epool.tile([128, EP], f32, tag="den")
            rcp_t = epool.tile([128, EP], f32, tag="rcp")
            scr_t = epool.tile([128, EP], f32, tag="scr")
            red_t = epool.tile([128, 1], f32, tag="red")
            if ep in ("fused", "stt", "ttr"):
                use_stt = ep in ("fused", "stt")
                use_ttr = ep in ("fused", "ttr")
                if use_stt:
                    # den = (ps + var) - num
                    nc.vector.scalar_tensor_tensor(
                        out=den_t[:K, :], in0=ps[:, :EP], scalar=var_t[:K],
                        in1=num_t[:K, :],
                        op0=mybir.AluOpType.add, op1=mybir.AluOpType.subtract,
                    )
                    nc.vector.scalar_tensor_tensor(
                        out=den_t[K:, :], in0=ps[:, EP:], scalar=var_t[K:],
                        in1=num_t[K:, :],
                        op0=mybir.AluOpType.add, op1=mybir.AluOpType.subtract,
                    )
                else:
                    nc.vector.tensor_scalar_add(den_t[:K, :], ps[:, :EP], var_t[:K])
                    nc.vector.tensor_scalar_add(den_t[K:, :], ps[:, EP:], var_t[K:])
                    nc.vector.tensor_tensor(
                        den_t[:], den_t[:], num_t[:], op=mybir.AluOpType.subtract
                    )
                nc.vector.reciprocal_approx_accurate(rcp_t[:], den_t[:], scr_t[:])
                if use_ttr:
                    # scr = (num * rcp) * (-1/K);  red = sum_free(scr)
                    nc.vector.tensor_tensor_reduce(
                        out=scr_t[:], in0=num_t[:], in1=rcp_t[:],
                        scale=-1.0 / K, scalar=0.0,
                        op0=mybir.AluOpType.mult, op1=mybir.AluOpType.add,
                        accum_out=red_t[:],
                    )
                else:
                    nc.vector.tensor_tensor(
                        scr_t[:], num_t[:], rcp_t[:], op=mybir.AluOpType.mult
                    )
                    nc.vector.tensor_reduce(
                        red_t[:], scr_t[:], axis=mybir.AxisListType.X,
                        op=mybir.AluOpType.add,
                    )
                    nc.vector.tensor_scalar_mul(red_t[:], red_t[:], -1.0 / K)
            else:
                nc.vector.tensor_scalar_add(den_t[:K, :], ps[:, :EP], var_t[:K])
                nc.vector.tensor_scalar_add(den_t[K:, :], ps[:, EP:], var_t[K:])
                nc.vector.tensor_tensor(
                    den_t[:], den_t[:], num_t[:], op=mybir.AluOpType.subtract
                )
                nc.vector.reciprocal_approx_accurate(rcp_t[:], den_t[:], scr_t[:])
                nc.vector.tensor_tensor(
                    scr_t[:], num_t[:], rcp_t[:], op=mybir.AluOpType.mult
                )
                nc.vector.tensor_reduce(
                    red_t[:], scr_t[:], axis=mybir.AxisListType.X,
                    op=mybir.AluOpType.add,
                )
                nc.vector.tensor_scalar_mul(red_t[:], red_t[:], -1.0 / K)
            if fin == "pe":
                # cross-partition reduce on PE: [1,1] = ones.T @ red
                ps1 = pspool.tile([1, 1], f32, tag="ps1")
                nc.tensor.matmul(ps1[:], red_t[:], ones_t[:], start=True, stop=True)
                out_sb = epool.tile([1, 1], f32, tag="out_sb")
                nc.vector.tensor_copy(out_sb[:], ps1[:])
                nc.scalar.dma_start(out_d[:], out_sb[:])
            else:
                nc.scalar.dma_start(out_d[:], red_t[:])

    nc.compile()
    return nc


def _prep_inputs(zs, X, var_noise, mm_dtype_name="float16"):
    """Host-side shard + layout packing (layout + dtype cast only; the
    only math is extracting diag(X))."""
    np_mm = {"float16": np.float16, "bfloat16": None, "float32r": np.float32,
             "float32": np.float32}[mm_dtype_name]
    if np_mm is None:
        import ml_dtypes
        np_mm = ml_dtypes.bfloat16
    zs = np.ascontiguousarray(np.asarray(zs, dtype=np.float32))
    X = np.ascontiguousarray(np.asarray(X, dtype=np.float32))
    var = np.float32(np.asarray(var_noise).reshape(()))

    # xt_packed[c, g, p, j*SHARD + il] = X[c*SHARD + il, (g*CPG + j)*128 + p]
    xt_packed = np.ascontiguousarray(
        X.reshape(NCORES, SHARD, GROUPS, CPG, 128)
        .transpose(0, 2, 4, 3, 1)
        .astype(np_mm)
    ).reshape(NCORES, GROUPS, 128, CPG * SHARD)

    # zst_pieces[g, p, j*K + k] = zs[k, (g*CPG + j)*128 + p]  (replicated)
    zst_pieces = np.ascontiguousarray(
        zs.reshape(K, GROUPS, CPG, 128).transpose(1, 3, 2, 0).astype(np_mm)
    ).reshape(GROUPS, 128, CPG * K)

    diag = np.ascontiguousarray(np.diagonal(X))
    var_tile = np.full((128, 1), var, dtype=np.float32)

    def fold(a):  # [K, SHARD] -> [128, EP] epilogue layout
        return np.ascontiguousarray(np.concatenate([a[:, :EP], a[:, EP:]], axis=0))

    in_maps = []
    for c in range(NCORES):
        sl = slice(c * SHARD, (c + 1) * SHARD)
        zs_sh = zs[:, sl]
        diag_bc = np.broadcast_to(diag[sl], (K, SHARD))
        in_maps.append(
            {
                "xt": xt_packed[c],
                "zst": zst_pieces,
                "zs_sh": fold(zs_sh),
                "diag": fold(diag_bc),
                "var": var_tile,
            }
        )
    return in_maps


def _run(in_maps, mm_dtype_name="float16", warm=N_WARM, fin="pe", ep="stt",
         **run_kwargs):
    from concourse.bass_utils import run_bass_kernel_spmd

    key = ("nc", mm_dtype_name, warm, fin, ep)
    if key not in _CACHE:
        _CACHE[key] = _build(mm_dtype_name, warm=warm, fin=fin, ep=ep)
    nc = _CACHE[key]
    return run_bass_kernel_spmd(
        nc, in_maps, core_ids=list(range(NCORES)), **run_kwargs
    )


def kernel(zs, X, var_noise):
    in_maps = _prep_inputs(zs, X, var_noise)
    res = _run(in_maps).results
    total = np.float32(0.0)
    for c in range(NCORES):
        total += res[c]["out"].astype(np.float32).sum(dtype=np.float32)
    return np.float32(total)
